# revision 1
# baseline (speedup 1.0000x reference)
"""Trainium2 Bass kernel for nn_CLGF_GNNDrug (GCN+GIN drug GNN, 8 cores).

Data-parallel over graphs:
  - Host: split 4000 graphs into 8 contiguous node slices, append self-loops,
    sort per-core incoming edges by dst tile (uniform KC chunks/tile), build
    per-chunk one-hot edge->dst matrices (M_gin 0/1, M_gcn norm-weighted) and
    pre-gather layer-1 source rows from x (x is an input).
  - Device (one SPMD program): aggregation via PE matmuls
    aggT[feat,dst] += gathered[128e,feat].T @ M^T[128e,128dst] in PSUM; dense
    layers feature-major (nodes on the free dim); BN via free-axis reductions
    + [128,2] AllReduce; layer-2/3 gather tables are AllGathered node-major
    bf16 slices read back with per-chunk indirect DMAs; segment-max pooling =
    segmented max scan + ap_gather extraction.
"""
import os
import sys
import types

import numpy as np
import ml_dtypes


def _install_ntff_hook():
    try:
        from antenv.axon_hooks import get_axon_ntff_profile_hook  # noqa: F401
        return
    except ImportError:
        pass
    try:
        from trn_agent_boot.trn_boot import _ntff_profile_via_ctypes
        hook = _ntff_profile_via_ctypes("/opt/axon/libaxon_pjrt.so")
    except Exception:
        hook = None
    mod = types.ModuleType("antenv.axon_hooks")
    mod.get_axon_ntff_profile_hook = lambda: hook
    mod.set_axon_ntff_profile_hook = lambda h: None
    sys.modules["antenv.axon_hooks"] = mod


_install_ntff_hook()

import concourse.bass as bass
import concourse.bacc as bacc
import concourse.mybir as mybir
import concourse.tile as tile
from concourse.bass_utils import run_bass_kernel_spmd

N = 100000
E = 500000
NG = 4000
F_IN = 77
D = 128
BN_EPS = 1e-5
NC = 8
P = 128
GRP = 512
NEG = -1.0e30

dt = mybir.dt
BF = dt.bfloat16
F32 = dt.float32
bf16 = ml_dtypes.bfloat16
ADD, MUL = None, None  # set after mybir import below


# ============================= host preprocessing =============================

def prep(x, edge_index, batch):
    x = np.asarray(x, np.float32)
    src_all = np.asarray(edge_index[0], np.int64)
    dst_all = np.asarray(edge_index[1], np.int64)
    batch = np.asarray(batch, np.int64)

    gsizes = np.bincount(batch, minlength=NG)
    gstart = np.concatenate([[0], np.cumsum(gsizes)])
    cuts = [0]
    for c in range(1, NC):
        target = c * N // NC
        g = int(np.searchsorted(gstart, target))
        if g > 0 and abs(gstart[g - 1] - target) < abs(gstart[min(g, NG)] - target):
            g -= 1
        g = min(max(g, cuts[-1]), NG)
        cuts.append(g)
    cuts.append(NG)
    g0 = np.array(cuts[:-1]); g1 = np.array(cuts[1:])
    n0 = gstart[g0]; n1 = gstart[g1]
    ncore = (n1 - n0).astype(np.int64)

    S = int(np.ceil(ncore.max() / GRP) * GRP)
    T = S // P
    NT = NC * S
    Gc = (g1 - g0).astype(np.int64)
    G_pad = int(np.ceil((Gc.max() + 1) / 16) * 16)

    core_of = np.searchsorted(n1, np.arange(N), side="right")
    local = np.arange(N) - n0[core_of]
    pgid = (core_of * S + local).astype(np.int64)

    deg = 1.0 + np.bincount(dst_all, minlength=N).astype(np.float64)
    dinv = 1.0 / np.sqrt(deg)
    es = np.concatenate([src_all, np.arange(N)])
    ed = np.concatenate([dst_all, np.arange(N)])
    w_gcn = np.concatenate(
        [dinv[src_all] * dinv[dst_all], dinv * dinv]).astype(np.float32)

    ecore = core_of[ed]
    dloc = local[ed]
    tile_of_edge = dloc // P
    KC = 0
    for c in range(NC):
        cnt = np.bincount(tile_of_edge[ecore == c], minlength=T)
        KC = max(KC, int(np.ceil(cnt.max() / P)))
    C = KC * T

    src_idx = np.zeros((NC, P, C), np.int32)
    m_gin = np.zeros((NC, P, C * P), bf16)
    m_gcn = np.zeros((NC, P, C * P), bf16)
    for c in range(NC):
        m = np.where(ecore == c)[0]
        t = tile_of_edge[m]
        order = np.argsort(t, kind="stable")
        m = m[order]; t = t[order]
        cnt = np.bincount(t, minlength=T)
        slot_base = np.concatenate([[0], np.cumsum(cnt)])[:-1]
        slot = np.arange(len(m)) - slot_base[t]
        j = t * KC + slot // P
        pp = slot % P
        src_idx[c, pp, j] = pgid[es[m]]
        f = dloc[m] % P
        m_gin[c, pp, j * P + f] = 1.0
        m_gcn[c, pp, j * P + f] = w_gcn[m].astype(bf16)

    x_pad = np.zeros((NT, D), bf16)
    x_pad[pgid, :F_IN] = x.astype(bf16)
    x_edges = np.zeros((NC, P, C * D), bf16)
    for c in range(NC):
        x_edges[c] = x_pad[src_idx[c]].reshape(P, C * D)

    resets = np.zeros((NC, S), np.float32)
    end_ids = np.zeros((NC, G_pad), np.int64)
    for c in range(NC):
        gs = gstart[g0[c]:g1[c] + 1] - n0[c]
        starts = gs[:-1]; ends = gs[1:] - 1
        ne = gsizes[g0[c]:g1[c]] > 0
        resets[c, starts[ne]] = NEG
        if ncore[c] < S:
            resets[c, ncore[c]] = NEG
        end_ids[c, :g1[c] - g0[c]][ne] = ends[ne]

    cnt = np.zeros((NC, S), np.float32)
    for c in range(NC):
        cnt[c, :ncore[c]] = deg[n0[c]:n1[c]].astype(np.float32)

    return dict(
        S=S, T=T, NT=NT, KC=KC, C=C, G_pad=G_pad,
        g0=g0, g1=g1, n0=n0, n1=n1, ncore=ncore, Gc=Gc, pgid=pgid,
        src_idx=src_idx, m_gin=m_gin, m_gcn=m_gcn, x_edges=x_edges,
        resets=resets, end_ids=end_ids, empty=(gsizes == 0), cnt=cnt,
    )


# ============================= device program =============================

def build_program(S, T, KC, C, G_pad):
    NT = NC * S
    nc = bacc.Bacc("TRN2", target_bir_lowering=False)
    AluOp = mybir.AluOpType
    Act = mybir.ActivationFunctionType

    x_edges = nc.dram_tensor("x_edges", [P, C * D], BF, kind="ExternalInput")
    src_idx = nc.dram_tensor("src_idx", [P, C], dt.int32, kind="ExternalInput")
    mgc = nc.dram_tensor("mgc", [P, C * P], BF, kind="ExternalInput")
    mgi = nc.dram_tensor("mgi", [P, C * P], BF, kind="ExternalInput")
    wstk = nc.dram_tensor("wstk", [8 * P, D], F32, kind="ExternalInput")
    pvec = nc.dram_tensor("pvec", [P, 14], F32, kind="ExternalInput")
    resets = nc.dram_tensor("resets", [P, S], BF, kind="ExternalInput")
    endi = nc.dram_tensor("endi", [P, G_pad // 16], dt.int16, kind="ExternalInput")
    bncor = nc.dram_tensor("bncor", [P, 6], F32, kind="ExternalInput")
    ident = nc.dram_tensor("ident", [P, P], BF, kind="ExternalInput")
    identf = nc.dram_tensor("identf", [P, P], F32, kind="ExternalInput")
    cntv = nc.dram_tensor("cntv", [1, S], F32, kind="ExternalInput")

    out = nc.dram_tensor("out", [G_pad * 9, D], F32, kind="ExternalOutput")

    # internal DRAM
    ag2_in = nc.dram_tensor("ag2_in", [S, 2 * D], BF)
    tab2 = nc.dram_tensor("tab2", [NT, 2 * D], BF, addr_space="Shared")
    ag3_in = nc.dram_tensor("ag3_in", [S, D], BF)
    tab3 = nc.dram_tensor("tab3", [NT, D], BF, addr_space="Shared")
    bn_in = [nc.dram_tensor(f"bn{i}_in", [P, 2], F32) for i in range(3)]
    bn_out = [nc.dram_tensor(f"bn{i}_out", [P, 2], F32, addr_space="Shared")
              for i in range(3)]
    # feature-major slice spill buffers [P, S] bf16
    sl_xg1 = nc.dram_tensor("sl_xg1", [P, S], BF)
    sl_xg2 = nc.dram_tensor("sl_xg2", [P, S], BF)
    sl_h1 = nc.dram_tensor("sl_h1", [P, S], BF)
    sl_h2 = nc.dram_tensor("sl_h2", [P, S], BF)
    sl_h3 = nc.dram_tensor("sl_h3", [P, S], BF)
    sl_u = [nc.dram_tensor(f"sl_u{i}", [P, S], BF) for i in range(3)]

    RG = [list(range(NC))]
    NGRP = S // GRP
    GPT = GRP // P

    with tile.TileContext(nc) as tc:
        with (
            tc.tile_pool(name="cst", bufs=1) as cst,
            tc.tile_pool(name="big", bufs=1) as big,
            tc.tile_pool(name="gat", bufs=7) as gat,
            tc.tile_pool(name="mbuf", bufs=6) as mbp,
            tc.tile_pool(name="work", bufs=3) as wkp,
            tc.tile_pool(name="scn", bufs=2) as scp,
            tc.tile_pool(name="ps", bufs=2, space="PSUM") as psp,
            tc.tile_pool(name="psd", bufs=1, space="PSUM") as psd,
            tc.tile_pool(name="pst", bufs=1, space="PSUM") as pst,
        ):
            # ---------------- constants ----------------
            w_sb = cst.tile([P, 8 * D], F32)
            for i in range(8):
                nc.sync.dma_start(out=w_sb[:, i * D:(i + 1) * D],
                                  in_=wstk[i * P:(i + 1) * P, :])
            pv = cst.tile([P, 14], F32)
            nc.sync.dma_start(out=pv[:], in_=pvec[:, :])
            idx_sb = cst.tile([P, C], dt.int32)
            nc.sync.dma_start(out=idx_sb[:], in_=src_idx[:, :])
            rst = big.tile([P, S], BF)
            nc.sync.dma_start(out=rst[:], in_=resets[:, :])
            endi_sb = cst.tile([P, G_pad // 16], dt.int16)
            nc.sync.dma_start(out=endi_sb[:], in_=endi[:, :])
            idn = cst.tile([P, P], BF)
            nc.sync.dma_start(out=idn[:], in_=ident[:, :])
            idnf = cst.tile([P, P], F32)
            nc.sync.dma_start(out=idnf[:], in_=identf[:, :])
            bnc = cst.tile([P, 6], F32)
            nc.sync.dma_start(out=bnc[:], in_=bncor[:, :])
            aeff_sb = cst.tile([P, 2 * D], F32)  # scaled A for layers 2,3
            atv_row = cst.tile([1, 2 * P], F32)  # (A^T t) rows for layers 2,3

            W1, W2 = w_sb[:, 0:D], w_sb[:, D:2 * D]
            A = [w_sb[:, (2 + 2 * i) * D:(3 + 2 * i) * D] for i in range(3)]
            B = [w_sb[:, (3 + 2 * i) * D:(4 + 2 * i) * D] for i in range(3)]
            b1c, b2c = pv[:, 0:1], pv[:, 1:2]
            ac = [pv[:, 2 + 4 * i:3 + 4 * i] for i in range(3)]
            cc = [pv[:, 3 + 4 * i:4 + 4 * i] for i in range(3)]
            gcl = [pv[:, 4 + 4 * i:5 + 4 * i] for i in range(3)]
            bec = [pv[:, 5 + 4 * i:6 + 4 * i] for i in range(3)]

            scanbuf = big.tile([P, S], F32)
            pooled = big.tile([P, G_pad * 9], F32)
            stats = cst.tile([P, 3 * 2 * NGRP], F32)
            sf = cst.tile([P, 8], F32)

            # =========== helpers ===========
            def agg_tiles(source, mats):
                """Generator over dst tiles.

                source: None -> x_edges stream (row=D); (table, row) -> indirect
                mats: list of (mdram, tag, feat_off)
                yields (t, [psum tiles [128, 128]])
                """
                row = D if source is None else source[1]
                for t in range(T):
                    if source is None:
                        g = gat.tile([P, KC * row], BF, tag="g")
                        nc.sync.dma_start(
                            out=g[:], in_=x_edges[:, t * KC * row:(t + 1) * KC * row])
                    else:
                        table = source[0]
                        g = gat.tile([P, KC * row], BF, tag="g")
                        for j in range(KC):
                            nc.gpsimd.indirect_dma_start(
                                out=g[:, j * row:(j + 1) * row],
                                out_offset=None,
                                in_=table[:, :],
                                in_offset=bass.IndirectOffsetOnAxis(
                                    ap=idx_sb[:, t * KC + j:t * KC + j + 1],
                                    axis=0),
                            )
                    mts = []
                    for (mdram, tag, fo) in mats:
                        mt = mbp.tile([P, KC * P], BF, tag=f"m{tag}")
                        nc.sync.dma_start(
                            out=mt[:], in_=mdram[:, t * KC * P:(t + 1) * KC * P])
                        mts.append(mt)
                    psums = [psp.tile([P, P], F32, tag=f"ps{tag}",
                                      name=f"ps_{tag}_{t}")
                             for (_, tag, _) in mats]
                    for j in range(KC):
                        for (mt, ps, (_, tag, fo)) in zip(mts, psums, mats):
                            nc.tensor.matmul(
                                ps[:], g[:, j * row + fo:j * row + fo + P],
                                mt[:, j * P:(j + 1) * P],
                                start=(j == 0), stop=(j == KC - 1))
                    yield t, psums

            def dense_gcn(aggbuf, W, bcol, dest_dram, gidx):
                ps = psd.tile([P, GRP], F32, tag="zd")
                nc.tensor.matmul(ps[:], W, aggbuf[:], start=True, stop=True)
                o = wkp.tile([P, GRP], BF, tag="obf")
                nc.scalar.activation(o[:], ps[:], Act.Relu, bias=bcol)
                nc.sync.dma_start(
                    out=dest_dram[:, gidx * GRP:(gidx + 1) * GRP], in_=o[:])
                return o

            def dense_gin(aggbuf, li, gidx, udest_dram, emit=None):
                ps1 = psd.tile([P, GRP], F32, tag="zd")
                if li == 0:
                    nc.tensor.matmul(ps1[:], A[0], aggbuf[:],
                                     start=True, stop=True)
                else:
                    Aeff = aeff_sb[:, (li - 1) * D:li * D]
                    nc.tensor.matmul(ps1[:], Aeff, aggbuf[:],
                                     start=True, stop=False)
                    cg = wkp.tile([1, GRP], F32, tag="cntg",
                                  name=f"cg{li}_{gidx}")
                    nc.sync.dma_start(
                        out=cg[:], in_=cntv[0:1, gidx * GRP:(gidx + 1) * GRP])
                    nc.tensor.matmul(
                        ps1[:], atv_row[0:1, (li - 1) * P:li * P],
                        cg[0:1, :], start=False, stop=True)
                ua = wkp.tile([P, GRP], F32, tag="ua")
                nc.scalar.activation(ua[:], ps1[:], Act.Relu, bias=ac[li])
                ps2 = psd.tile([P, GRP], F32, tag="zd2")
                nc.tensor.matmul(ps2[:], B[li], ua[:], start=True, stop=True)
                o = wkp.tile([P, GRP], BF, tag="obf")
                base = li * 2 * NGRP
                nc.scalar.activation(o[:], ps2[:], Act.Relu, bias=cc[li],
                                     accum_out=stats[:, base + 2 * gidx:
                                                     base + 2 * gidx + 1])
                nc.sync.dma_start(
                    out=udest_dram[:, gidx * GRP:(gidx + 1) * GRP], in_=o[:])
                if emit is not None:
                    ag_dest, col0 = emit
                    for kk in range(GPT):
                        emit_nm(o, kk, ag_dest, gidx * GPT + kk, col0)
                sq = wkp.tile([P, GRP], F32, tag="sq")
                nc.vector.tensor_tensor(out=sq[:], in0=o[:], in1=o[:],
                                        op=AluOp.mult)
                nc.vector.reduce_sum(
                    stats[:, base + 2 * gidx + 1:base + 2 * gidx + 2], sq[:],
                    axis=mybir.AxisListType.X)

            def bn_finalize(li):
                base = li * 2 * NGRP
                nc.vector.reduce_sum(sf[:, 0:1], stats[:, base:base + 2 * NGRP:2],
                                     axis=mybir.AxisListType.X)
                nc.vector.reduce_sum(sf[:, 1:2],
                                     stats[:, base + 1:base + 2 * NGRP:2],
                                     axis=mybir.AxisListType.X)
                nc.vector.tensor_tensor(out=sf[:, 0:2], in0=sf[:, 0:2],
                                        in1=bnc[:, 2 * li:2 * li + 2],
                                        op=AluOp.subtract)
                nc.sync.dma_start(out=bn_in[li][:, :], in_=sf[:, 0:2])
                nc.gpsimd.collective_compute(
                    "AllReduce", AluOp.add, replica_groups=RG,
                    ins=[bn_in[li][:, :]], outs=[bn_out[li][:, :]])
                nc.sync.dma_start(out=sf[:, 2:4], in_=bn_out[li][:, :])
                nc.vector.tensor_scalar(out=sf[:, 4:5], in0=sf[:, 2:3],
                                        scalar1=1.0 / N, scalar2=None,
                                        op0=AluOp.mult)
                nc.vector.tensor_scalar(out=sf[:, 5:6], in0=sf[:, 3:4],
                                        scalar1=1.0 / N, scalar2=None,
                                        op0=AluOp.mult)
                nc.vector.tensor_tensor(out=sf[:, 6:7], in0=sf[:, 4:5],
                                        in1=sf[:, 4:5], op=AluOp.mult)
                nc.vector.tensor_tensor(out=sf[:, 5:6], in0=sf[:, 5:6],
                                        in1=sf[:, 6:7], op=AluOp.subtract)
                nc.vector.tensor_scalar(out=sf[:, 5:6], in0=sf[:, 5:6],
                                        scalar1=BN_EPS, scalar2=None,
                                        op0=AluOp.add)
                nc.scalar.activation(sf[:, 5:6], sf[:, 5:6], Act.Sqrt)
                nc.vector.reciprocal(sf[:, 6:7], sf[:, 5:6])
                nc.vector.tensor_tensor(out=sf[:, 6:7], in0=sf[:, 6:7],
                                        in1=gcl[li], op=AluOp.mult)
                nc.vector.tensor_tensor(out=sf[:, 7:8], in0=sf[:, 4:5],
                                        in1=sf[:, 6:7], op=AluOp.mult)
                nc.vector.tensor_tensor(out=sf[:, 7:8], in0=bec[li],
                                        in1=sf[:, 7:8], op=AluOp.subtract)
                if li < 2:
                    # A_eff = diag(s) @ A[li+1] ; atv = (A[li+1]^T @ t) as row
                    nc.vector.tensor_scalar(
                        out=aeff_sb[:, li * D:(li + 1) * D], in0=A[li + 1],
                        scalar1=sf[:, 6:7], scalar2=None, op0=AluOp.mult)
                    pv1 = pst.tile([P, 1], F32, tag="po", name=f"atv{li}")
                    nc.tensor.matmul(pv1[:], A[li + 1], sf[:, 7:8],
                                     start=True, stop=True)
                    atc = wkp.tile([P, 1], F32, tag="nm", name=f"atc{li}")
                    nc.vector.tensor_copy(atc[:], pv1[:])
                    pv2 = pst.tile([1, P], F32, tag="po", name=f"atr{li}")
                    nc.tensor.transpose(pv2[:], atc[:], idnf[:])
                    nc.vector.tensor_copy(atv_row[0:1, li * P:(li + 1) * P],
                                          pv2[:])
                return sf[:, 6:7], sf[:, 7:8]

            def emit_nm(src_tile, k, ag_dest, t, col0):
                ptile = pst.tile([P, P], BF, tag="tp")
                nc.tensor.transpose(ptile[:], src_tile[:, k * P:(k + 1) * P],
                                    idn[:])
                nm = wkp.tile([P, P], BF, tag="nm")
                nc.vector.tensor_copy(nm[:], ptile[:])
                nc.sync.dma_start(
                    out=ag_dest[t * P:(t + 1) * P, col0:col0 + P], in_=nm[:])

            def scan_comp(comp_idx, srcs, op=None, bn=None, bn3=None):
                prev = None
                for gi in range(NGRP):
                    tiles = []
                    for si_, sdram in enumerate(srcs):
                        st = scp.tile([P, GRP], BF, tag="cin",
                                      name=f"cin{comp_idx}_{gi}_{si_}")
                        nc.sync.dma_start(
                            out=st[:], in_=sdram[:, gi * GRP:(gi + 1) * GRP])
                        tiles.append(st)
                    if bn is not None:
                        ht = scp.tile([P, GRP], F32, tag="cmp",
                                      name=f"hb{comp_idx}_{gi}")
                        nc.scalar.activation(ht[:], tiles[0][:], Act.Identity,
                                             bias=bn[1], scale=bn[0])
                        data1 = ht[:]
                    elif bn3 is not None:
                        hts = []
                        for ii in range(3):
                            h_ = scp.tile([P, GRP], F32, tag=f"h3_{ii}",
                                          name=f"h3_{comp_idx}_{gi}_{ii}")
                            nc.scalar.activation(h_[:], tiles[ii][:],
                                                 Act.Identity,
                                                 bias=bn3[ii][1],
                                                 scale=bn3[ii][0])
                            hts.append(h_)
                        tmp = scp.tile([P, GRP], F32, tag="cmp",
                                       name=f"c3_{comp_idx}_{gi}")
                        nc.vector.tensor_tensor(out=tmp[:], in0=hts[0][:],
                                                in1=hts[1][:], op=op)
                        nc.vector.tensor_tensor(out=tmp[:], in0=tmp[:],
                                                in1=hts[2][:], op=op)
                        data1 = tmp[:]
                    elif len(tiles) == 1:
                        data1 = tiles[0][:]
                    else:
                        tmp = scp.tile([P, GRP], F32, tag="cmp",
                                       name=f"c2_{comp_idx}_{gi}")
                        nc.vector.tensor_tensor(out=tmp[:], in0=tiles[0][:],
                                                in1=tiles[1][:], op=op)
                        data1 = tmp[:]
                    init = 0.0 if prev is None else prev
                    nc.vector.tensor_tensor_scan(
                        out=scanbuf[:, gi * GRP:(gi + 1) * GRP],
                        data0=rst[:, gi * GRP:(gi + 1) * GRP],
                        data1=data1,
                        initial=init,
                        op0=AluOp.add,
                        op1=AluOp.max)
                    prev = scanbuf[:, (gi + 1) * GRP - 1:(gi + 1) * GRP]
                ext = scp.tile([P, G_pad], F32, tag="ext",
                               name=f"ext{comp_idx}")
                nc.gpsimd.ap_gather(
                    out_ap=ext[:].rearrange("p (g o) -> p g o", o=1),
                    in_ap=scanbuf[:].rearrange("p (s o) -> p s o", o=1),
                    idxs_ap=endi_sb[:], channels=P, num_elems=S, d=1,
                    num_idxs=G_pad)
                nc.vector.tensor_copy(pooled[:, comp_idx::9], ext[:])

            # ================= LAYER 1 =================
            agg_gc = wkp.tile([P, GRP], F32, tag="agc")
            agg_gi = wkp.tile([P, GRP], F32, tag="agi")
            for t, (psg, psi) in agg_tiles(None, [(mgc, "gc", 0),
                                                  (mgi, "gi", 0)]):
                k = t % GPT
                nc.scalar.copy(agg_gc[:, k * P:(k + 1) * P], psg[:])
                nc.vector.tensor_copy(agg_gi[:, k * P:(k + 1) * P], psi[:])
                if k == GPT - 1:
                    gi = t // GPT
                    xg1t = dense_gcn(agg_gc, W1, b1c, sl_xg1, gi)
                    for kk in range(GPT):
                        emit_nm(xg1t, kk, ag2_in, gi * GPT + kk, 0)
                    dense_gin(agg_gi, 0, gi, sl_u[0], emit=(ag2_in, P))
                    agg_gc = wkp.tile([P, GRP], F32, tag="agc")
                    agg_gi = wkp.tile([P, GRP], F32, tag="agi")

            nc.gpsimd.collective_compute(
                "AllGather", AluOp.bypass, replica_groups=RG,
                ins=[ag2_in[:, :]], outs=[tab2[:, :]])
            s1, t1 = bn_finalize(0)
            s1s = cst.tile([P, 2], F32, name="s1s")
            nc.vector.tensor_copy(s1s[:], sf[:, 6:8])

            scan_comp(0, [sl_xg1])
            scan_comp(4, [sl_u[0]], bn=(s1s[:, 0:1], s1s[:, 1:2]))

            # ================= LAYER 2 =================
            agg_gc = wkp.tile([P, GRP], F32, tag="agc")
            agg_gi = wkp.tile([P, GRP], F32, tag="agi")
            for t, (psg, psi) in agg_tiles((tab2, 2 * D),
                                           [(mgc, "gc", 0), (mgi, "gi", P)]):
                k = t % GPT
                nc.scalar.copy(agg_gc[:, k * P:(k + 1) * P], psg[:])
                nc.vector.tensor_copy(agg_gi[:, k * P:(k + 1) * P], psi[:])
                if k == GPT - 1:
                    gi = t // GPT
                    dense_gcn(agg_gc, W2, b2c, sl_xg2, gi)
                    dense_gin(agg_gi, 1, gi, sl_u[1], emit=(ag3_in, 0))
                    agg_gc = wkp.tile([P, GRP], F32, tag="agc")
                    agg_gi = wkp.tile([P, GRP], F32, tag="agi")

            nc.gpsimd.collective_compute(
                "AllGather", AluOp.bypass, replica_groups=RG,
                ins=[ag3_in[:, :]], outs=[tab3[:, :]])
            s2, t2 = bn_finalize(1)
            s2s = cst.tile([P, 2], F32, name="s2s")
            nc.vector.tensor_copy(s2s[:], sf[:, 6:8])

            scan_comp(1, [sl_xg2])
            scan_comp(2, [sl_xg1, sl_xg2], AluOp.add)
            scan_comp(3, [sl_xg1, sl_xg2], AluOp.mult)
            scan_comp(5, [sl_u[1]], bn=(s2s[:, 0:1], s2s[:, 1:2]))

            # ================= LAYER 3 =================
            agg_gi = wkp.tile([P, GRP], F32, tag="agi")
            for t, (psi,) in agg_tiles((tab3, D), [(mgi, "gi", 0)]):
                k = t % GPT
                nc.vector.tensor_copy(agg_gi[:, k * P:(k + 1) * P], psi[:])
                if k == GPT - 1:
                    gi = t // GPT
                    dense_gin(agg_gi, 2, gi, sl_u[2])
                    agg_gi = wkp.tile([P, GRP], F32, tag="agi")

            s3, t3 = bn_finalize(2)
            s3s = cst.tile([P, 2], F32, name="s3s")
            nc.vector.tensor_copy(s3s[:], sf[:, 6:8])

            scan_comp(6, [sl_u[2]], bn=(s3s[:, 0:1], s3s[:, 1:2]))
            bns3 = [(s1s[:, 0:1], s1s[:, 1:2]), (s2s[:, 0:1], s2s[:, 1:2]),
                    (s3s[:, 0:1], s3s[:, 1:2])]
            scan_comp(7, [sl_u[0], sl_u[1], sl_u[2]], AluOp.add, bn3=bns3)
            scan_comp(8, [sl_u[0], sl_u[1], sl_u[2]], AluOp.mult, bn3=bns3)

            NPT = (G_pad * 9 + P - 1) // P
            for t in range(NPT):
                c0 = t * P
                w = min(P, G_pad * 9 - c0)
                ptile = pst.tile([P, P], F32, tag="po")
                nc.tensor.transpose(ptile[:w, :], pooled[:, c0:c0 + w], idnf[:])
                nm = scp.tile([P, P], F32, tag="pon")
                nc.vector.tensor_copy(nm[:w, :], ptile[:w, :])
                nc.sync.dma_start(out=out[c0:c0 + w, :], in_=nm[:w, :])

    nc.finalize()
    return nc


# ============================= top-level kernel =============================

_CACHE = {}


def kernel(x, edge_index, batch, W1, b1, W2, b2,
           A0, a0, B0, c0, g0, be0,
           A1, a1, B1, c1, g1, be1,
           A2, a2, B2, c2, g2, be2):
    pp = prep(x, edge_index, batch)
    S, T, KC, C, G_pad = pp["S"], pp["T"], pp["KC"], pp["C"], pp["G_pad"]

    key = (S, T, KC, C, G_pad)
    if key not in _CACHE:
        _CACHE[key] = build_program(S, T, KC, C, G_pad)
    nc = _CACHE[key]

    def pad_w(W):
        Wp = np.zeros((P, D), np.float32)
        W = np.asarray(W, np.float32)
        Wp[:W.shape[0]] = W
        return Wp

    wstk = np.concatenate([pad_w(W1), pad_w(W2), pad_w(A0), pad_w(B0),
                           pad_w(A1), pad_w(B1), pad_w(A2), pad_w(B2)], axis=0)
    pvec = np.stack([np.asarray(v, np.float32) for v in
                     (b1, b2, a0, c0, g0, be0, a1, c1, g1, be1,
                      a2, c2, g2, be2)], axis=1)
    ident = np.eye(P, dtype=bf16)
    identf = np.eye(P, dtype=np.float32)

    total_pads = NC * S - N
    bncor = np.zeros((P, 6), np.float32)
    for li, (Aw, av, Bw, cv) in enumerate(
            ((A0, a0, B0, c0), (A1, a1, B1, c1), (A2, a2, B2, c2))):
        ua = np.maximum(np.asarray(av, np.float32), 0.0)
        u_pad = np.maximum(ua @ np.asarray(Bw, np.float32)
                           + np.asarray(cv, np.float32), 0.0)
        bncor[:, 2 * li] = total_pads * u_pad
        bncor[:, 2 * li + 1] = total_pads * u_pad * u_pad

    in_maps = []
    for c in range(NC):
        ends = pp["end_ids"][c].astype(np.int16)
        endw = ends.reshape(G_pad // 16, 16).T.copy()
        endw = np.tile(endw, (8, 1))
        in_maps.append(dict(
            x_edges=pp["x_edges"][c],
            src_idx=pp["src_idx"][c],
            mgc=pp["m_gcn"][c],
            mgi=pp["m_gin"][c],
            wstk=wstk,
            pvec=pvec.astype(np.float32),
            resets=np.tile(pp["resets"][c][None, :], (P, 1)).astype(bf16),
            endi=endw,
            bncor=bncor,
            ident=ident,
            identf=identf,
            cntv=pp["cnt"][c][None, :],
        ))

    trace = bool(os.environ.get("KERNEL_TRACE"))
    res = run_bass_kernel_spmd(nc, in_maps, list(range(NC)), trace=trace)
    kernel.last_exec_ns = res.exec_time_ns
    kernel.last_result = res

    outp = np.zeros((NG, 9 * D), np.float32)
    for c in range(NC):
        oc = res.results[c]["out"].reshape(G_pad, 9 * D)
        Gc = pp["g1"][c] - pp["g0"][c]
        outp[pp["g0"][c]:pp["g1"][c]] = oc[:Gc]
    outp[pp["empty"]] = -np.inf
    return outp

